# revision 16
# baseline (speedup 1.0000x reference)
"""Butterfly (10-stage, n=1024) as a dense composed matmul on 8 TRN2 cores.

Strategy:
  - Host: compose the 10 butterfly stage matrices into one dense W
    (1024x1024, f64 accumulate -> f32). out = x @ W^T + bias.
  - Host: pack x into PE-friendly transposed tiles so every DMA is a
    contiguous 512KB read with 4KB partition lines:
        xt[tile][c'][j][b] = x[128*tile + b, 128*j + c']
  - Device (per core, 4096 rows = 32 tiles): for each tile, 16
    accumulating matmuls (lhsT = xt chunk [c'=128, b=128] stationary,
    rhs = W^T chunk [c'=128, n=512] moving, fp32r dtype -> 1 cycle/row),
    then DVE adds bias (replicated across partitions) while moving
    PSUM->SBUF, then DMA out (contiguous 512KB).
  - Data-parallel over batch: core k handles rows [4096k, 4096(k+1)).
"""

import numpy as np

import concourse.bass as bass
import concourse.bacc as bacc
import concourse.mybir as mybir
from concourse.tile import TileContext
from concourse.bass_utils import run_bass_kernel_spmd

N_CORES = 8
BATCH = 32768
NPOS = 1024
NSTAGE = 10
P = 128
NCHUNK = NPOS // P  # 8
TILES_PER_CORE = BATCH // N_CORES // P  # 32

MM_DTYPE = mybir.dt.float32r


def _compose_w(twiddle: np.ndarray) -> np.ndarray:
    """Compose the butterfly stages into M_id[c, n] = W[n, c] (= W^T).

    Applies the reference butterfly to the identity matrix in float64.
    Row c of the result is B @ e_c, i.e. column c of the composed W.
    """
    tw = np.asarray(twiddle, dtype=np.float64)  # (1, 10, 512, 2, 2)
    n = NPOS
    out = np.eye(n, dtype=np.float64).reshape(n, 1, n)  # (batch=n, nstack=1, n)
    for idx in range(NSTAGE):
        stride = 1 << idx
        nb = n // (2 * stride)
        t = tw[:, idx].reshape(1, nb, stride, 2, 2).transpose(0, 1, 3, 4, 2)
        o = out.reshape(n, 1, nb, 1, 2, stride)
        out = (t * o).sum(axis=4).reshape(n, 1, n)
    return out.reshape(n, n)  # [c, n]


def _build_nc(precision_probe: bool = False, repeats: int = 1) -> bass.Bass:
    nc = bacc.Bacc()
    f32 = mybir.dt.float32

    xt = nc.declare_dram_parameter(
        "xt", [TILES_PER_CORE, P, NCHUNK, P], MM_DTYPE, isOutput=False
    )
    w = nc.declare_dram_parameter("w", [P, NCHUNK, NPOS], MM_DTYPE, isOutput=False)
    bias = nc.declare_dram_parameter("bias", [P, NPOS], f32, isOutput=False)
    out = nc.declare_dram_parameter(
        "out", [TILES_PER_CORE, P, NPOS], f32, isOutput=True
    )
    if precision_probe:
        out_f32 = nc.declare_dram_parameter("out_f32", [P, NPOS], f32, isOutput=True)
        out_bf16 = nc.declare_dram_parameter("out_bf16", [P, NPOS], f32, isOutput=True)

    with TileContext(nc) as tc:
        with (
            tc.tile_pool(name="const", bufs=1) as cpool,
            tc.tile_pool(name="xtp", bufs=3) as xpool,
            tc.tile_pool(name="outp", bufs=3) as opool,
            tc.tile_pool(name="ps", bufs=4, space="PSUM") as pspool,
            tc.tile_pool(name="psprobe", bufs=2, space="PSUM") as prpool,
        ):
            w_sb = cpool.tile([P, NCHUNK, NPOS], MM_DTYPE)
            nc.sync.dma_start(out=w_sb[:], in_=w[:])
            b_sb = cpool.tile([P, NPOS], f32)
            nc.sync.dma_start(out=b_sb[:], in_=bias[:])

            for _rep in range(repeats):
              for t in range(TILES_PER_CORE):
                xt_sb = xpool.tile([P, NCHUNK, P], MM_DTYPE)
                nc.sync.dma_start(out=xt_sb[:], in_=xt[t])
                o_sb = opool.tile([P, NPOS], f32)
                for nh in range(2):
                    ns = nh * 512
                    ps = pspool.tile([P, 512], f32)
                    for j in range(NCHUNK):
                        nc.tensor.matmul(
                            ps[:],
                            lhsT=xt_sb[:, j, :],
                            rhs=w_sb[:, j, ns : ns + 512],
                            start=(j == 0),
                            stop=(j == NCHUNK - 1),
                        )
                    nc.vector.tensor_add(
                        out=o_sb[:, ns : ns + 512],
                        in0=ps[:],
                        in1=b_sb[:, ns : ns + 512],
                    )
                nc.sync.dma_start(out=out[t], in_=o_sb[:])

                if precision_probe and t == 0 and _rep == 0:
                    # true-f32 operand tiles for the probe variants
                    xt_32 = xpool.tile([P, NCHUNK, P], f32, tag="probex32")
                    nc.sync.dma_start(out=xt_32[:], in_=xt[0].bitcast(f32))
                    w_32 = cpool.tile([P, NCHUNK, NPOS], f32)
                    nc.sync.dma_start(out=w_32[:], in_=w[:].bitcast(f32))

                    # fp32 (regular, 4 cyc/row) variant of tile 0
                    o32 = opool.tile([P, NPOS], f32, tag="probe32")
                    for nh in range(2):
                        ns = nh * 512
                        ps = prpool.tile([P, 512], f32, tag="pspr")
                        for j in range(NCHUNK):
                            nc.tensor.matmul(
                                ps[:],
                                lhsT=xt_32[:, j, :],
                                rhs=w_32[:, j, ns : ns + 512],
                                start=(j == 0),
                                stop=(j == NCHUNK - 1),
                            )
                        nc.vector.tensor_add(
                            out=o32[:, ns : ns + 512], in0=ps[:], in1=b_sb[:, ns : ns + 512]
                        )
                    nc.sync.dma_start(out=out_f32[:], in_=o32[:])

                    # bf16 variant of tile 0
                    bf = mybir.dt.bfloat16
                    xt_bf = xpool.tile([P, NCHUNK, P], bf, tag="probexbf")
                    nc.vector.tensor_copy(out=xt_bf[:], in_=xt_32[:])
                    w_bf = cpool.tile([P, NCHUNK, NPOS], bf)
                    nc.vector.tensor_copy(out=w_bf[:], in_=w_32[:])
                    obf = opool.tile([P, NPOS], f32, tag="probebf")
                    for nh in range(2):
                        ns = nh * 512
                        ps = prpool.tile([P, 512], f32, tag="pspr")
                        for j in range(NCHUNK):
                            nc.tensor.matmul(
                                ps[:],
                                lhsT=xt_bf[:, j, :],
                                rhs=w_bf[:, j, ns : ns + 512],
                                start=(j == 0),
                                stop=(j == NCHUNK - 1),
                            )
                        nc.vector.tensor_add(
                            out=obf[:, ns : ns + 512], in0=ps[:], in1=b_sb[:, ns : ns + 512]
                        )
                    nc.sync.dma_start(out=out_bf16[:], in_=obf[:])
    nc.compile()
    return nc


def _pack_inputs(x: np.ndarray, twiddle: np.ndarray, bias: np.ndarray):
    x = np.asarray(x, dtype=np.float32)
    bias = np.asarray(bias, dtype=np.float32)

    m_id = _compose_w(twiddle).astype(np.float32)  # [c, n] = W^T
    w_packed = np.ascontiguousarray(
        m_id.reshape(NCHUNK, P, NPOS).transpose(1, 0, 2)
    )  # [c', j, n]
    bias_rep = np.ascontiguousarray(np.broadcast_to(bias, (P, NPOS)))

    # [ntile, c', j, b] with ntile = 256 global tiles of 128 rows
    xt_all = np.ascontiguousarray(
        x.reshape(BATCH // P, P, NCHUNK, P).transpose(0, 3, 2, 1)
    )
    return xt_all, w_packed, bias_rep


def kernel(
    x: np.ndarray,
    twiddle: np.ndarray,
    bias: np.ndarray,
    _trace: bool = False,
    _probe: bool = False,
):
    xt_all, w_packed, bias_rep = _pack_inputs(x, twiddle, bias)

    nc = _build_nc(precision_probe=_probe)
    in_maps = []
    for k in range(N_CORES):
        in_maps.append(
            {
                "xt": xt_all[k * TILES_PER_CORE : (k + 1) * TILES_PER_CORE],
                "w": w_packed,
                "bias": bias_rep,
            }
        )
    res = run_bass_kernel_spmd(nc, in_maps, list(range(N_CORES)), trace=_trace)

    out = np.concatenate(
        [np.asarray(r["out"]).reshape(-1, NPOS) for r in res.results], axis=0
    ).astype(np.float32, copy=False)

    if _trace or _probe:
        kernel.last_results = res
    return out
